# revision 9
# baseline (speedup 1.0000x reference)
"""Trainium2 Bass kernel for nn_ContextEncoderLayer — v2, folded attention.

Shards L across 8 cores (LC=256/core).  Algebraic folds kill the big K/V
projections (34 of 39 GFLOP/core):
  scores(l,d,h) = X[l,d,:] . r[l,h,:]   with  r = Wk_h @ (src@Wq+bq)/8
  ctx(l,h,:)    = (sum_d probs*X[l,d,:]) @ Wv_h + bv
bk is softmax-invariant (constant over d) and dropped; bv folded into src
host-side; bq folded via the ones-row of st.

Per (l,d)-tile t (128 rows = 4 positions x 32 candidates):
  scores: 8 MMs, stationary = X^T chunk (FWL), moving = r-block [128,64]
  mask-add (-1e30 off-diag) -> exp -> probs (zeros kill cross-l terms)
  den: ones-row MM  [1,64]; u~^T: 8 MMs, stationary = X row chunk
  ctx: per-(h,c) MMs with strided uT slices; den via DRAM-roundtrip reshape.
FFN unchanged in math; W1/W2 used as stationary operands (FWL).
"""

import sys

sys.path.insert(0, "/opt/trn_rl_repo")

from contextlib import ExitStack

import numpy as np
import ml_dtypes

import concourse.bacc as bacc
import concourse.tile as tile
from concourse import mybir
from concourse.bass_utils import run_bass_kernel_spmd
from concourse.masks import make_identity

L, D, DM, H, FF = 2048, 32, 1024, 16, 4096
DH = DM // H  # 64
SCALE = float(np.sqrt(DH))  # 8.0
NCORES = 8
LC = L // NCORES  # 256 positions per core
NT = LC * D // 128  # 64 (l,d)-row tiles per core
NLT = LC // 128  # 2 l-tiles per core
NC_DM = DM // 128  # 8 dm chunks
CH = NC_DM + 1  # 9: 8 chunks + ones-row (bq fold)
NFF = FF // 128  # 32 ff chunks
TPB = NT // NLT  # 32 (l,d)-tiles per l-tile
NHP = H // 2  # 8 head pairs
BF = mybir.dt.bfloat16
F8 = mybir.dt.float8e4
F32 = mybir.dt.float32

_CACHE = {}

# Route Exp/Ln to the combined natural_log_exp_and_others table set so the
# softmax exp and the LN rsqrt (exp(-0.5*ln(var))) share one resident table
# (set ids keep their act_info.json positions, only eligibility changes).
import concourse.bacc as _bacc_mod

_orig_gat = _bacc_mod.get_activation_tables


def _gat_combined(arch):
    t = dict(_orig_gat(arch))
    AFt = mybir.ActivationFunctionType
    if "natural_log_exp_and_others" in t:
        for name, fns in t.items():
            if name != "natural_log_exp_and_others":
                fns = set(fns)
                fns.discard(AFt.Exp)
                fns.discard(AFt.Ln)
                t[name] = fns
    return t


_bacc_mod.get_activation_tables = _gat_combined


def _build_nc(repeat=1):
    nc = bacc.Bacc("TRN2", target_bir_lowering=False, debug=False, num_devices=NCORES)

    # ---------------- I/O ----------------
    xx_in = nc.dram_tensor("xx", [NT, 128, 2048], F8, kind="ExternalInput")
    st_in = nc.dram_tensor("st", [128, CH * LC], BF, kind="ExternalInput")
    wq_in = nc.dram_tensor("wq", [128, CH * 1024], BF, kind="ExternalInput")
    wkt_in = nc.dram_tensor("wkt", [128, NHP * NC_DM * 128], BF, kind="ExternalInput")
    wvn_in = nc.dram_tensor("wvn", [128, NC_DM * 1024], BF, kind="ExternalInput")
    ww_in = nc.dram_tensor("ww", [NFF, 128, 2048], BF, kind="ExternalInput")
    maskp_in = nc.dram_tensor("maskp", [NLT, 128, TPB * 64], F32, kind="ExternalInput")
    srcb_in = nc.dram_tensor("srcb", [LC, DM], F32, kind="ExternalInput")
    b1_in = nc.dram_tensor("b1p", [128, NFF], F32, kind="ExternalInput")
    b2_in = nc.dram_tensor("b2p", [1, DM], F32, kind="ExternalInput")
    g1_in = nc.dram_tensor("g1p", [1, DM], F32, kind="ExternalInput")
    be1_in = nc.dram_tensor("be1p", [1, DM], F32, kind="ExternalInput")
    g2_in = nc.dram_tensor("g2p", [1, DM], F32, kind="ExternalInput")
    be2_in = nc.dram_tensor("be2p", [1, DM], F32, kind="ExternalInput")
    out = nc.dram_tensor("out", [LC, DM], F32, kind="ExternalOutput")

    AL = mybir.AluOpType
    AF = mybir.ActivationFunctionType

    with tile.TileContext(nc) as tc, ExitStack() as top:
        consts = top.enter_context(tc.tile_pool(name="consts", bufs=1))
        dram = top.enter_context(tc.tile_pool(name="dram", bufs=1, space="DRAM"))

        # ------- params resident in SBUF -------
        wvn_sb = consts.tile([128, NC_DM * 1024], BF)
        nc.gpsimd.dma_start(wvn_sb[:], wvn_in[:])
        b1_sb = consts.tile([128, NFF], F32)
        nc.gpsimd.dma_start(b1_sb[:], b1_in[:])
        ident = consts.tile([128, 128], BF)
        make_identity(nc, ident[:])
        eps_sb = consts.tile([128, 1], F32)
        nc.vector.memset(eps_sb[:], 1e-5)
        ones_sb = consts.tile([128, 1], BF)
        nc.vector.memset(ones_sb[:], 1.0)

        def rep128(name, src):  # [1, DM] -> [128, DM] partition-broadcast
            t = consts.tile([128, DM], F32, name=name)
            nc.gpsimd.dma_start(t[:], src[0:1, :].broadcast_to([128, DM]))
            return t

        g1_rep = rep128("g1_rep", g1_in)
        be1_rep = rep128("be1_rep", be1_in)
        g2_rep = rep128("g2_rep", g2_in)
        be2_rep = rep128("be2_rep", be2_in)
        b2_rep = rep128("b2_rep", b2_in)

        qrt_pool = top.enter_context(tc.tile_pool(name="qrt", bufs=1))
        utp = top.enter_context(tc.tile_pool(name="utp", bufs=1))
        xres = top.enter_context(tc.tile_pool(name="xres", bufs=1))
        xtp = top.enter_context(tc.tile_pool(name="xtp", bufs=1))

        for _rep in range(repeat):
            # =========== phase A: qT = (Wq' @ srcT)/8 ; rT = Wk_h @ qT_h ===========
            rt_sb = qrt_pool.tile(
                [128, NC_DM * H * LC], BF, name=f"rt{_rep}", tag="rt"
            )
            pha = ExitStack()
            apool = pha.enter_context(tc.tile_pool(name="apool", bufs=1))
            st_sb = apool.tile([128, CH * LC], BF, name=f"st{_rep}", tag="st")
            nc.sync.dma_start(st_sb[:], st_in[:])
            wq_sb = apool.tile([128, CH * 1024], BF, name=f"wqs{_rep}", tag="wqs")
            nc.sync.dma_start(wq_sb[:], wq_in[:])
            wkt_sb = apool.tile([128, NHP * NC_DM * 128], BF, name=f"wkts{_rep}", tag="wkts")
            nc.sync.dma_start(wkt_sb[:], wkt_in[:])
            qt_sb = apool.tile([128, NHP * LC], BF, name=f"qt{_rep}", tag="qt")
            with tc.tile_pool(name="qps", bufs=3, space="PSUM") as qpsp:
                for hp in range(NHP):
                    qps = qpsp.tile([128, LC], F32, name=f"qps{_rep}_{hp}", tag="qps")
                    for c in range(CH):
                        if c < NC_DM:
                            lhsT = wq_sb[:, c * 1024 + hp * 128 : c * 1024 + hp * 128 + 128]
                            rhs = st_sb[:, c * LC : (c + 1) * LC]
                        else:
                            lhsT = wq_sb[0:1, c * 1024 + hp * 128 : c * 1024 + hp * 128 + 128]
                            rhs = st_sb[0:1, c * LC : (c + 1) * LC]
                        nc.tensor.matmul(
                            qps[:], lhsT, rhs, start=(c == 0), stop=(c == CH - 1)
                        )
                    nc.scalar.copy(qt_sb[:, hp * LC : (hp + 1) * LC], qps[:])

            rt_v0 = rt_sb.rearrange("p (c h l) -> p c h l", c=NC_DM, h=H)
            rt_v4 = rt_sb.rearrange("p (c h l) -> p c h l", c=NC_DM, h=H)
            with tc.tile_pool(name="rps", bufs=4, space="PSUM") as rpsp:
                for hp in range(NHP):
                    for half in range(2):
                        rp2 = []
                        for hl in range(2):
                            rp2.append(rpsp.tile(
                                [128, 4 * LC], F32,
                                name=f"rps{_rep}_{hp}_{half}_{hl}", tag="rps",
                            ))
                        # interleave hl=0 (rows 0-63) / hl=1 (rows 64-127) MMs
                        # so the PE runs the two row-groups concurrently
                        for cc in range(4):
                            c = half * 4 + cc
                            for hl in range(2):
                                lhsT = wkt_sb[
                                    hl * 64 : hl * 64 + 64,
                                    hp * 1024 + c * 128 : hp * 1024 + (c + 1) * 128,
                                ]
                                rhs = qt_sb[
                                    hl * 64 : hl * 64 + 64, hp * LC : (hp + 1) * LC
                                ]
                                nc.tensor.matmul(
                                    rp2[hl][:, cc * LC : (cc + 1) * LC], lhsT, rhs,
                                    start=True, stop=True,
                                )
                        for hl in range(2):
                            h = 2 * hp + hl
                            dst = rt_v4[:, half * 4 : half * 4 + 4, h, :]
                            srcv = rp2[hl][:].rearrange("p (c l) -> p c l", c=4)
                            if hl == 1:
                                nc.vector.tensor_copy(dst, srcv)
                            else:
                                nc.scalar.copy(dst, srcv)

            pha.close()

            # rt view for per-tile slices: [p, c, h, l]
            rt_v = rt_sb.rearrange("p (c h l) -> p c h l", c=NC_DM, h=H)

            # =========== phase B: per l-tile attention ===========
            x_tiles = []
            xT_sb = xtp.tile([128, NC_DM * LC], BF, name=f"xT{_rep}", tag="xT")
            den_dram = dram.tile([NLT, TPB * 64], F32, name=f"dend{_rep}", tag="dend")
            with ExitStack() as pb:
                sc_psp = pb.enter_context(tc.tile_pool(name="sc_ps", bufs=2, space="PSUM"))
                u_psp = pb.enter_context(tc.tile_pool(name="u_ps", bufs=2, space="PSUM"))
                den_psp = pb.enter_context(tc.tile_pool(name="den_ps", bufs=2, space="PSUM"))
                ctx_psp = pb.enter_context(tc.tile_pool(name="ctx_ps", bufs=1, space="PSUM"))
                xc_pool = pb.enter_context(tc.tile_pool(name="xc_pool", bufs=6))
                mk_pool = pb.enter_context(tc.tile_pool(name="mk_pool", bufs=1))
                sc_pool = pb.enter_context(tc.tile_pool(name="sc_pool", bufs=4))
                den_pool = pb.enter_context(tc.tile_pool(name="den_pool", bufs=1))
                ln_pool = pb.enter_context(tc.tile_pool(name="ln_pool", bufs=1))
                s2_pool = pb.enter_context(tc.tile_pool(name="s2_pool", bufs=2))

                for lt in range(NLT):
                    ut_sb = utp.tile(
                        [128, TPB * 512], BF, name=f"ut{_rep}_{lt}", tag="ut"
                    )
                    ut_v = ut_sb.rearrange("p (c h l) -> p c h l", c=NC_DM, h=H)
                    den_sb = den_pool.tile(
                        [1, TPB * 64], F32, name=f"den{_rep}_{lt}", tag="den"
                    )
                    mk_lt = mk_pool.tile([128, TPB * 64], F32, name=f"{_rep}mk{lt}", tag="mk")
                    nc.gpsimd.dma_start(mk_lt[:], maskp_in[lt])
                    for tt in range(TPB):
                        t = lt * TPB + tt
                        xx_t = xc_pool.tile([128, 2048], F8, name=f"{_rep}xx{t}", tag="xx")
                        nc.sync.dma_start(xx_t[:], xx_in[t])
                        xc_t = xx_t[:, 0:1024]
                        xr_t = xx_t[:, 1024:2048]
                        mk_t = mk_lt[:, tt * 64 : (tt + 1) * 64]

                        # scores: [128 (i,d), 64 (i',h)]
                        scps = sc_psp.tile([128, 64], F32, name=f"{_rep}scp{t}", tag="scp")
                        for c in range(NC_DM):
                            rhs = rt_v[:, c, :, 4 * t : 4 * t + 4]
                            nc.tensor.matmul(
                                scps[:],
                                xc_t[:, c * 128 : (c + 1) * 128],
                                rhs,
                                start=(c == 0),
                                stop=(c == NC_DM - 1),
                            )
                        sc_sb = sc_pool.tile([128, 64], F32, name=f"{_rep}scs{t}", tag="scs")
                        nc.vector.tensor_tensor(sc_sb[:], scps[:], mk_t, AL.add)
                        ex = sc_pool.tile([128, 64], BF, name=f"{_rep}ex{t}", tag="ex")
                        nc.scalar.activation(ex[:], sc_sb[:], AF.Exp)

                        # den: [1, 64] = column sums of ex
                        dps = den_psp.tile([1, 64], F32, name=f"{_rep}dp{t}", tag="dp")
                        nc.tensor.matmul(dps[:], ones_sb[:], ex[:], start=True, stop=True)
                        nc.scalar.copy(
                            den_sb[:, tt * 64 : (tt + 1) * 64].rearrange(
                                "o (i h) -> o i h", i=4
                            ),
                            dps[:].rearrange("o (h i) -> o i h", h=H),
                        )

                        # u~^T: [128 dm-in-c, (c, 64)]
                        ups = u_psp.tile([128, 512], F32, name=f"{_rep}up{t}", tag="up")
                        for c in range(NC_DM):
                            nc.tensor.matmul(
                                ups[:, c * 64 : (c + 1) * 64],
                                xr_t[:, c * 128 : (c + 1) * 128],
                                ex[:],
                                start=True,
                                stop=True,
                            )
                        ut_dst = ut_v[:, :, :, tt * 4 : (tt + 1) * 4]
                        ut_src = ups[:].rearrange("p (c h i) -> p c h i", c=NC_DM, h=H)
                        if tt % 2 == 0:
                            nc.scalar.copy(ut_dst, ut_src)
                        else:
                            nc.vector.tensor_copy(ut_dst, ut_src)

                    # --- den roundtrip: [1, (tt,i,h)] -> [128 (tt,i), 16 h] ---
                    nc.sync.dma_start(
                        den_dram[lt].rearrange("(o n) -> o n", o=1), den_sb[:]
                    )
                    den_lt = den_pool.tile([128, H], F32, name=f"{_rep}dl{lt}", tag="dl")
                    nc.sync.dma_start(
                        den_lt[:], den_dram[lt].rearrange("(p h) -> p h", h=H)
                    )
                    rd = den_pool.tile([128, H], F32, name=f"{_rep}rd{lt}", tag="rd")
                    nc.vector.reciprocal(rd[:], den_lt[:])

                    # --- ctx GEMM: [128 l, 1024 (h,dh)] ---
                    ctxps = ctx_psp.tile([128, 1024], F32, name=f"{_rep}cx{lt}", tag="cx")
                    for h in range(H):
                        for c in range(NC_DM):
                            lhsT = ut_v[:, c, h, :]
                            nc.tensor.matmul(
                                ctxps[:, h * 64 : (h + 1) * 64],
                                lhsT,
                                wvn_sb[:, c * 1024 + h * 64 : c * 1024 + (h + 1) * 64],
                                start=(c == 0),
                                stop=(c == NC_DM - 1),
                            )

                    # --- normalize + residual + LN1 ---
                    ctxn = ln_pool.tile([128, 1024], F32, name=f"{_rep}cn{lt}", tag="cn")
                    nc.vector.tensor_tensor(
                        ctxn.rearrange("p (h x) -> p h x", x=DH),
                        ctxps.rearrange("p (h x) -> p h x", x=DH),
                        rd.rearrange("p (h o) -> p h o", o=1).broadcast_to([128, H, DH]),
                        AL.mult,
                    )
                    src_sb = ln_pool.tile([128, 1024], F32, name=f"{_rep}sr{lt}", tag="sr")
                    nc.gpsimd.dma_start(src_sb[:], srcb_in[lt * 128 : (lt + 1) * 128, :])
                    r = ln_pool.tile([128, 1024], F32, name=f"{_rep}r{lt}", tag="r")
                    nc.vector.tensor_tensor(r[:], ctxn[:], src_sb[:], AL.add)
                    rsum = s2_pool.tile([128, 1], F32, name=f"{_rep}rs{lt}", tag="rs")
                    nc.vector.tensor_reduce(rsum[:], r[:], axis=mybir.AxisListType.X, op=AL.add)
                    nmean = s2_pool.tile([128, 1], F32, name=f"{_rep}mn{lt}", tag="mn")
                    nc.vector.tensor_scalar_mul(nmean[:], rsum[:], -1.0 / DM)
                    sq = ln_pool.tile([128, 1024], F32, name=f"{_rep}sq{lt}", tag="xm")
                    ssq = s2_pool.tile([128, 1], F32, name=f"{_rep}sm{lt}", tag="sm")
                    nc.scalar.activation(sq[:], r[:], AF.Square, bias=nmean[:], accum_out=ssq[:])
                    lnv = s2_pool.tile([128, 1], F32, name=f"{_rep}sd{lt}", tag="sd")
                    nc.scalar.activation(lnv[:], ssq[:], AF.Ln, bias=eps_sb[:], scale=1.0 / DM)
                    rstd = s2_pool.tile([128, 1], F32, name=f"{_rep}rsd{lt}", tag="rsd")
                    nc.scalar.activation(rstd[:], lnv[:], AF.Exp, scale=-0.5)
                    xn = ln_pool.tile([128, 1024], F32, name=f"{_rep}xn{lt}", tag="sr")
                    nc.vector.tensor_scalar(
                        out=xn[:], in0=r[:], scalar1=nmean[:], scalar2=rstd[:],
                        op0=AL.add, op1=AL.mult,
                    )
                    t1 = ln_pool.tile([128, 1024], F32, name=f"{_rep}t1{lt}", tag="cn")
                    nc.vector.tensor_tensor(t1[:], xn[:], g1_rep[:], AL.mult)
                    x = xres.tile([128, 1024], F32, name=f"x{_rep}_{lt}", tag=f"x{lt}")
                    x_tiles.append(x)
                    nc.vector.tensor_tensor(x[:], t1[:], be1_rep[:], AL.add)
                    x_bf = ln_pool.tile([128, 1024], BF, name=f"{_rep}xb{lt}", tag="xb")
                    nc.vector.tensor_copy(x_bf[:], x[:])
                    for c in range(NC_DM):
                        if True:
                            tp = den_psp.tile([128, 128], BF, name=f"{_rep}tp{lt}_{c}", tag="dp")
                            nc.tensor.transpose(tp[:], x_bf[:, c * 128 : (c + 1) * 128], ident[:])
                            nc.scalar.copy(
                                xT_sb[:, c * LC + lt * 128 : c * LC + (lt + 1) * 128], tp[:]
                            )

            # =========== phase C: FFN + LN2 ===========
            with ExitStack() as pc:
                ff_psp = pc.enter_context(tc.tile_pool(name="ff_ps", bufs=3, space="PSUM"))
                o_psp = pc.enter_context(tc.tile_pool(name="o_ps", bufs=1, space="PSUM"))
                w1_pool = pc.enter_context(tc.tile_pool(name="w1_pool", bufs=3))
                ff1_sb = utp.tile([128, NFF * LC], BF, name=f"ff1_{_rep}", tag="ut")
                outps = [
                    o_psp.tile([128, 512], F32, name=f"{_rep}ops{i}", tag=f"ops{i}")
                    for i in range(4)
                ]
                for cc in range(NFF):
                    wwt = w1_pool.tile([128, 2048], BF, name=f"{_rep}ww{cc}", tag="w1t")
                    nc.gpsimd.dma_start(wwt[:], ww_in[cc])
                    w1t = wwt[:, 0:1024]
                    ffps = ff_psp.tile([128, LC], F32, name=f"{_rep}ffps{cc}", tag="ffps")
                    for c in range(NC_DM):
                        nc.tensor.matmul(
                            ffps[:],
                            wwt[:, c * 128 : (c + 1) * 128],
                            xT_sb[:, c * LC : (c + 1) * LC],
                            start=(c == 0),
                            stop=(c == NC_DM - 1),
                        )
                    nc.scalar.activation(
                        ff1_sb[:, cc * LC : (cc + 1) * LC],
                        ffps[:],
                        AF.Gelu,
                        bias=b1_sb[:, cc : cc + 1],
                    )
                    w2t = wwt[:, 1024:2048]
                    for lt in range(NLT):
                        for hh in range(2):
                            nc.tensor.matmul(
                                outps[lt * 2 + hh][:],
                                ff1_sb[:, cc * LC + lt * 128 : cc * LC + (lt + 1) * 128],
                                wwt[:, 1024 + hh * 512 : 1024 + (hh + 1) * 512],
                                start=(cc == 0),
                                stop=(cc == NFF - 1),
                            )

                ln2_pool = pc.enter_context(tc.tile_pool(name="ln2_pool", bufs=2))
                s3_pool = pc.enter_context(tc.tile_pool(name="s3_pool", bufs=2))
                for lt in range(NLT):
                    xb2 = ln2_pool.tile([128, 1024], F32, name=f"{_rep}xb2_{lt}", tag="xb2")
                    nc.vector.tensor_tensor(xb2[:], x_tiles[lt][:], b2_rep[:], AL.add)
                    r2 = ln2_pool.tile([128, 1024], F32, name=f"{_rep}r2_{lt}", tag="r2")
                    for hh in range(2):
                        nc.vector.tensor_tensor(
                            r2[:, hh * 512 : (hh + 1) * 512],
                            xb2[:, hh * 512 : (hh + 1) * 512],
                            outps[lt * 2 + hh][:],
                            AL.add,
                        )
                    rsum2 = s3_pool.tile([128, 1], F32, name=f"{_rep}rsum2_{lt}", tag="rs2")
                    nc.vector.tensor_reduce(rsum2[:], r2[:], axis=mybir.AxisListType.X, op=AL.add)
                    nmean2 = s3_pool.tile([128, 1], F32, name=f"{_rep}mean2_{lt}", tag="mn2")
                    nc.vector.tensor_scalar_mul(nmean2[:], rsum2[:], -1.0 / DM)
                    sq2 = ln2_pool.tile([128, 1024], F32, name=f"{_rep}sq2_{lt}", tag="xm2")
                    ssq2 = s3_pool.tile([128, 1], F32, name=f"{_rep}ssq2_{lt}", tag="sm2")
                    nc.scalar.activation(sq2[:], r2[:], AF.Square, bias=nmean2[:], accum_out=ssq2[:])
                    lnv2 = s3_pool.tile([128, 1], F32, name=f"{_rep}std2_{lt}", tag="sd2")
                    nc.scalar.activation(lnv2[:], ssq2[:], AF.Ln, bias=eps_sb[:], scale=1.0 / DM)
                    rstd2 = s3_pool.tile([128, 1], F32, name=f"{_rep}rstd2_{lt}", tag="rsd2")
                    nc.scalar.activation(rstd2[:], lnv2[:], AF.Exp, scale=-0.5)
                    xn2 = ln2_pool.tile([128, 1024], F32, name=f"{_rep}xn2_{lt}", tag="xb2")
                    nc.vector.tensor_scalar(
                        out=xn2[:], in0=r2[:], scalar1=nmean2[:], scalar2=rstd2[:],
                        op0=AL.add, op1=AL.mult,
                    )
                    t2 = ln2_pool.tile([128, 1024], F32, name=f"{_rep}t2_{lt}", tag="r2")
                    nc.vector.tensor_tensor(t2[:], xn2[:], g2_rep[:], AL.mult)
                    y = ln2_pool.tile([128, 1024], F32, name=f"{_rep}y{lt}", tag="xm2")
                    nc.vector.tensor_tensor(y[:], t2[:], be2_rep[:], AL.add)
                    nc.sync.dma_start(out[lt * 128 : (lt + 1) * 128, :], y[:])

    nc.compile()
    return nc


def _prep_core(src_c, tgt_c, mask_c, W):
    """Host-side layout prep for one core's shard.  Returns the in_map dict."""
    bf = ml_dtypes.bfloat16
    X = np.ascontiguousarray(tgt_c.reshape(LC * D, DM))

    f8 = ml_dtypes.float8_e4m3
    # xx: [xc | xr]: xc = X^T chunks, xr = natural row tiles (fp8e4m3)
    xx = np.empty((NT, 128, 2048), dtype=f8)
    xx[:, :, 0:1024] = X.reshape(NT, 128, NC_DM, 128).transpose(0, 3, 2, 1).reshape(
        NT, 128, 1024
    ).astype(f8)
    xx[:, :, 1024:2048] = X.reshape(NT, 128, 1024).astype(f8)

    # st: [128, CH*LC]; [p, c*LC+f] = src_c[f, c*128+p]; chunk 8 row0 = ones
    st = np.zeros((128, CH, LC), dtype=np.float32)
    st[:, :NC_DM, :] = src_c.reshape(LC, NC_DM, 128).transpose(2, 1, 0)
    st[0, NC_DM, :] = 1.0

    def wprep(Wm, b, scale=1.0):
        Wp = np.zeros((CH * 128, DM), dtype=np.float32)
        Wp[:DM] = Wm * scale
        Wp[DM] = b * scale
        return np.ascontiguousarray(
            Wp.reshape(CH, 128, DM).transpose(1, 0, 2).reshape(128, CH * 1024)
        ).astype(bf)

    # wkt: [hl*64+p, hp*1024 + c*128 + m] = Wk[c*128+m, (2hp+hl)*64+p]
    wkt = np.ascontiguousarray(
        W["Wk"].reshape(NC_DM, 128, NHP, 2, 64).transpose(3, 4, 2, 0, 1).reshape(128, NHP * NC_DM * 128)
    ).astype(bf)

    wvn = np.ascontiguousarray(
        W["Wv"].reshape(NC_DM, 128, 1024).transpose(1, 0, 2).reshape(128, NC_DM * 1024)
    ).astype(bf)
    ww = np.empty((NFF, 128, 2048), dtype=bf)
    ww[:, :, 0:1024] = W["W1"].reshape(NC_DM, 128, NFF, 128).transpose(2, 1, 0, 3).reshape(
        NFF, 128, 1024
    ).astype(bf)
    ww[:, :, 1024:2048] = W["W2"].reshape(NFF, 128, DM).astype(bf)

    # maskp: [t, i*32+d, i2*16+h] = mask[4t+i, d] if i2==i else -1e30
    maskp = np.full((NT, 4, D, H, 4), -1e30, dtype=np.float32)
    mc = mask_c.reshape(NT, 4, D)
    for i in range(4):
        maskp[:, i, :, :, i] = mc[:, i, :, None]
    maskp = np.ascontiguousarray(
        maskp.reshape(NLT, TPB, 128, 64).transpose(0, 2, 1, 3).reshape(NLT, 128, TPB * 64)
    )

    srcb = np.ascontiguousarray(src_c + W["bv"][None, :]).astype(np.float32)

    return {
        "xx": xx,
        "st": np.ascontiguousarray(st.reshape(128, CH * LC)).astype(bf),
        "wq": wprep(W["Wq"], W["bq"], scale=1.0 / SCALE),
        "wkt": wkt,
        "wvn": wvn,
        "ww": ww,
        "maskp": maskp,
        "srcb": srcb,
        "b1p": np.ascontiguousarray(W["b1"].reshape(NFF, 128).T).astype(np.float32),
        "b2p": W["b2"].reshape(1, DM).astype(np.float32),
        "g1p": W["g1"].reshape(1, DM).astype(np.float32),
        "be1p": W["beta1"].reshape(1, DM).astype(np.float32),
        "g2p": W["g2"].reshape(1, DM).astype(np.float32),
        "be2p": W["beta2"].reshape(1, DM).astype(np.float32),
    }


def make_in_maps(**inputs):
    inp = {k: np.asarray(v) for k, v in inputs.items()}
    W = {
        k: inp[k]
        for k in ("Wq", "bq", "Wk", "bk", "Wv", "bv", "W1", "b1", "W2", "b2",
                  "g1", "beta1", "g2", "beta2")
    }
    in_maps = []
    for c in range(NCORES):
        sl = slice(c * LC, (c + 1) * LC)
        in_maps.append(_prep_core(inp["src"][sl], inp["target"][sl], inp["attn_mask"][sl], W))
    return in_maps


def get_nc(repeat=1):
    key = ("nc", repeat)
    if key not in _CACHE:
        _CACHE[key] = _build_nc(repeat)
    return _CACHE[key]


def kernel(**inputs) -> np.ndarray:
    nc = get_nc()
    in_maps = make_in_maps(**inputs)
    res = run_bass_kernel_spmd(nc, in_maps, core_ids=list(range(NCORES)))
    return np.concatenate([res.results[c]["out"] for c in range(NCORES)], axis=0)


# revision 10
# speedup vs baseline: 1.0308x; 1.0308x over previous
"""Trainium2 Bass kernel for nn_ContextEncoderLayer — v2, folded attention.

Shards L across 8 cores (LC=256/core).  Algebraic folds kill the big K/V
projections (34 of 39 GFLOP/core):
  scores(l,d,h) = X[l,d,:] . r[l,h,:]   with  r = Wk_h @ (src@Wq+bq)/8
  ctx(l,h,:)    = (sum_d probs*X[l,d,:]) @ Wv_h + bv
bk is softmax-invariant (constant over d) and dropped; bv folded into src
host-side; bq folded via the ones-row of st.

Per (l,d)-tile t (128 rows = 4 positions x 32 candidates):
  scores: 8 MMs, stationary = X^T chunk (FWL), moving = r-block [128,64]
  mask-add (-1e30 off-diag) -> exp -> probs (zeros kill cross-l terms)
  den: ones-row MM  [1,64]; u~^T: 8 MMs, stationary = X row chunk
  ctx: per-(h,c) MMs with strided uT slices; den via DRAM-roundtrip reshape.
FFN unchanged in math; W1/W2 used as stationary operands (FWL).
"""

import sys

sys.path.insert(0, "/opt/trn_rl_repo")

from contextlib import ExitStack

import numpy as np
import ml_dtypes

import concourse.bacc as bacc
import concourse.tile as tile
from concourse import mybir
from concourse.bass_utils import run_bass_kernel_spmd
from concourse.masks import make_identity

L, D, DM, H, FF = 2048, 32, 1024, 16, 4096
DH = DM // H  # 64
SCALE = float(np.sqrt(DH))  # 8.0
NCORES = 8
LC = L // NCORES  # 256 positions per core
NT = LC * D // 128  # 64 (l,d)-row tiles per core
NLT = LC // 128  # 2 l-tiles per core
NC_DM = DM // 128  # 8 dm chunks
CH = NC_DM + 1  # 9: 8 chunks + ones-row (bq fold)
NFF = FF // 128  # 32 ff chunks
TPB = NT // NLT  # 32 (l,d)-tiles per l-tile
NHP = H // 2  # 8 head pairs
BF = mybir.dt.bfloat16
F8 = mybir.dt.float8e4
F32 = mybir.dt.float32

_CACHE = {}

def _build_nc(repeat=1):
    nc = bacc.Bacc("TRN2", target_bir_lowering=False, debug=False, num_devices=NCORES)

    # ---------------- I/O ----------------
    xx_in = nc.dram_tensor("xx", [NT, 128, 2048], F8, kind="ExternalInput")
    st_in = nc.dram_tensor("st", [128, CH * LC], BF, kind="ExternalInput")
    wq_in = nc.dram_tensor("wq", [128, CH * 1024], BF, kind="ExternalInput")
    wkt_in = nc.dram_tensor("wkt", [128, NHP * NC_DM * 128], BF, kind="ExternalInput")
    wvn_in = nc.dram_tensor("wvn", [128, NC_DM * 1024], BF, kind="ExternalInput")
    ww_in = nc.dram_tensor("ww", [NFF, 128, 2048], BF, kind="ExternalInput")
    maskp_in = nc.dram_tensor("maskp", [NLT, 128, TPB * 64], F32, kind="ExternalInput")
    srcb_in = nc.dram_tensor("srcb", [LC, DM], F32, kind="ExternalInput")
    b1_in = nc.dram_tensor("b1p", [128, NFF], F32, kind="ExternalInput")
    b2_in = nc.dram_tensor("b2p", [1, DM], F32, kind="ExternalInput")
    g1_in = nc.dram_tensor("g1p", [1, DM], F32, kind="ExternalInput")
    be1_in = nc.dram_tensor("be1p", [1, DM], F32, kind="ExternalInput")
    g2_in = nc.dram_tensor("g2p", [1, DM], F32, kind="ExternalInput")
    be2_in = nc.dram_tensor("be2p", [1, DM], F32, kind="ExternalInput")
    out = nc.dram_tensor("out", [LC, DM], F32, kind="ExternalOutput")

    AL = mybir.AluOpType
    AF = mybir.ActivationFunctionType

    with tile.TileContext(nc) as tc, ExitStack() as top:
        consts = top.enter_context(tc.tile_pool(name="consts", bufs=1))
        dram = top.enter_context(tc.tile_pool(name="dram", bufs=1, space="DRAM"))

        # ------- params resident in SBUF -------
        wvn_sb = consts.tile([128, NC_DM * 1024], BF)
        nc.gpsimd.dma_start(wvn_sb[:], wvn_in[:])
        b1_sb = consts.tile([128, NFF], F32)
        nc.gpsimd.dma_start(b1_sb[:], b1_in[:])
        ident = consts.tile([128, 128], BF)
        make_identity(nc, ident[:])
        eps_sb = consts.tile([128, 1], F32)
        nc.vector.memset(eps_sb[:], 1e-5)
        ones_sb = consts.tile([128, 1], BF)
        nc.vector.memset(ones_sb[:], 1.0)

        def rep128(name, src):  # [1, DM] -> [128, DM] partition-broadcast
            t = consts.tile([128, DM], F32, name=name)
            nc.gpsimd.dma_start(t[:], src[0:1, :].broadcast_to([128, DM]))
            return t

        g1_rep = rep128("g1_rep", g1_in)
        be1_rep = rep128("be1_rep", be1_in)
        g2_rep = rep128("g2_rep", g2_in)
        be2_rep = rep128("be2_rep", be2_in)
        b2_rep = rep128("b2_rep", b2_in)

        qrt_pool = top.enter_context(tc.tile_pool(name="qrt", bufs=1))
        utp = top.enter_context(tc.tile_pool(name="utp", bufs=1))
        xres = top.enter_context(tc.tile_pool(name="xres", bufs=1))
        xtp = top.enter_context(tc.tile_pool(name="xtp", bufs=1))

        for _rep in range(repeat):
            # =========== phase A: qT = (Wq' @ srcT)/8 ; rT = Wk_h @ qT_h ===========
            rt_sb = qrt_pool.tile(
                [128, NC_DM * H * LC], BF, name=f"rt{_rep}", tag="rt"
            )
            pha = ExitStack()
            apool = pha.enter_context(tc.tile_pool(name="apool", bufs=1))
            st_sb = apool.tile([128, CH * LC], BF, name=f"st{_rep}", tag="st")
            nc.sync.dma_start(st_sb[:], st_in[:])
            wq_sb = apool.tile([128, CH * 1024], BF, name=f"wqs{_rep}", tag="wqs")
            nc.sync.dma_start(wq_sb[:], wq_in[:])
            wkt_sb = apool.tile([128, NHP * NC_DM * 128], BF, name=f"wkts{_rep}", tag="wkts")
            nc.sync.dma_start(wkt_sb[:], wkt_in[:])
            qt_sb = apool.tile([128, NHP * LC], BF, name=f"qt{_rep}", tag="qt")
            with tc.tile_pool(name="qps", bufs=3, space="PSUM") as qpsp:
                for hp in range(NHP):
                    qps = qpsp.tile([128, LC], F32, name=f"qps{_rep}_{hp}", tag="qps")
                    for c in range(CH):
                        if c < NC_DM:
                            lhsT = wq_sb[:, c * 1024 + hp * 128 : c * 1024 + hp * 128 + 128]
                            rhs = st_sb[:, c * LC : (c + 1) * LC]
                        else:
                            lhsT = wq_sb[0:1, c * 1024 + hp * 128 : c * 1024 + hp * 128 + 128]
                            rhs = st_sb[0:1, c * LC : (c + 1) * LC]
                        nc.tensor.matmul(
                            qps[:], lhsT, rhs, start=(c == 0), stop=(c == CH - 1)
                        )
                    nc.scalar.copy(qt_sb[:, hp * LC : (hp + 1) * LC], qps[:])

            rt_v0 = rt_sb.rearrange("p (c h l) -> p c h l", c=NC_DM, h=H)
            with tc.tile_pool(name="rps", bufs=2, space="PSUM") as rpsp:
                for h in range(H):
                    hp, hl = h // 2, h % 2
                    rps = rpsp.tile(
                        [128, NC_DM * LC], F32, name=f"rps{_rep}_{h}", tag="rps"
                    )
                    for c in range(NC_DM):
                        lhsT = wkt_sb[
                            hl * 64 : hl * 64 + 64,
                            hp * 1024 + c * 128 : hp * 1024 + (c + 1) * 128,
                        ]
                        rhs = qt_sb[hl * 64 : hl * 64 + 64, hp * LC : (hp + 1) * LC]
                        nc.tensor.matmul(
                            rps[:, c * LC : (c + 1) * LC], lhsT, rhs,
                            start=True, stop=True,
                        )
                    # rt layout: [p, c*(H*LC) + h*LC + l] — strided dst over c
                    if h % 3 == 2:
                        nc.vector.tensor_copy(
                            rt_v0[:, :, h, :],
                            rps[:].rearrange("p (c l) -> p c l", c=NC_DM),
                        )
                    else:
                        nc.scalar.copy(
                            rt_v0[:, :, h, :],
                            rps[:].rearrange("p (c l) -> p c l", c=NC_DM),
                        )

            pha.close()

            # rt view for per-tile slices: [p, c, h, l]
            rt_v = rt_sb.rearrange("p (c h l) -> p c h l", c=NC_DM, h=H)

            # =========== phase B: per l-tile attention ===========
            x_tiles = []
            xT_sb = xtp.tile([128, NC_DM * LC], BF, name=f"xT{_rep}", tag="xT")
            den_dram = dram.tile([NLT, TPB * 64], F32, name=f"dend{_rep}", tag="dend")
            with ExitStack() as pb:
                sc_psp = pb.enter_context(tc.tile_pool(name="sc_ps", bufs=2, space="PSUM"))
                u_psp = pb.enter_context(tc.tile_pool(name="u_ps", bufs=2, space="PSUM"))
                den_psp = pb.enter_context(tc.tile_pool(name="den_ps", bufs=2, space="PSUM"))
                ctx_psp = pb.enter_context(tc.tile_pool(name="ctx_ps", bufs=1, space="PSUM"))
                xc_pool = pb.enter_context(tc.tile_pool(name="xc_pool", bufs=6))
                mk_pool = pb.enter_context(tc.tile_pool(name="mk_pool", bufs=1))
                sc_pool = pb.enter_context(tc.tile_pool(name="sc_pool", bufs=4))
                den_pool = pb.enter_context(tc.tile_pool(name="den_pool", bufs=1))
                ln_pool = pb.enter_context(tc.tile_pool(name="ln_pool", bufs=1))
                s2_pool = pb.enter_context(tc.tile_pool(name="s2_pool", bufs=2))

                for lt in range(NLT):
                    ut_sb = utp.tile(
                        [128, TPB * 512], BF, name=f"ut{_rep}_{lt}", tag="ut"
                    )
                    ut_v = ut_sb.rearrange("p (c h l) -> p c h l", c=NC_DM, h=H)
                    den_sb = den_pool.tile(
                        [1, TPB * 64], F32, name=f"den{_rep}_{lt}", tag="den"
                    )
                    mk_lt = mk_pool.tile([128, TPB * 64], F32, name=f"{_rep}mk{lt}", tag="mk")
                    nc.gpsimd.dma_start(mk_lt[:], maskp_in[lt])
                    for tt in range(TPB):
                        t = lt * TPB + tt
                        xx_t = xc_pool.tile([128, 2048], F8, name=f"{_rep}xx{t}", tag="xx")
                        nc.sync.dma_start(xx_t[:], xx_in[t])
                        xc_t = xx_t[:, 0:1024]
                        xr_t = xx_t[:, 1024:2048]
                        mk_t = mk_lt[:, tt * 64 : (tt + 1) * 64]

                        # scores: [128 (i,d), 64 (i',h)]
                        scps = sc_psp.tile([128, 64], F32, name=f"{_rep}scp{t}", tag="scp")
                        for c in range(NC_DM):
                            rhs = rt_v[:, c, :, 4 * t : 4 * t + 4]
                            nc.tensor.matmul(
                                scps[:],
                                xc_t[:, c * 128 : (c + 1) * 128],
                                rhs,
                                start=(c == 0),
                                stop=(c == NC_DM - 1),
                            )
                        sc_sb = sc_pool.tile([128, 64], F32, name=f"{_rep}scs{t}", tag="scs")
                        nc.vector.tensor_tensor(sc_sb[:], scps[:], mk_t, AL.add)
                        ex = sc_pool.tile([128, 64], BF, name=f"{_rep}ex{t}", tag="ex")
                        nc.scalar.activation(ex[:], sc_sb[:], AF.Exp)

                        # den: [1, 64] = column sums of ex
                        dps = den_psp.tile([1, 64], F32, name=f"{_rep}dp{t}", tag="dp")
                        nc.tensor.matmul(dps[:], ones_sb[:], ex[:], start=True, stop=True)
                        nc.scalar.copy(
                            den_sb[:, tt * 64 : (tt + 1) * 64].rearrange(
                                "o (i h) -> o i h", i=4
                            ),
                            dps[:].rearrange("o (h i) -> o i h", h=H),
                        )

                        # u~^T: [128 dm-in-c, (c, 64)]
                        ups = u_psp.tile([128, 512], F32, name=f"{_rep}up{t}", tag="up")
                        for c in range(NC_DM):
                            nc.tensor.matmul(
                                ups[:, c * 64 : (c + 1) * 64],
                                xr_t[:, c * 128 : (c + 1) * 128],
                                ex[:],
                                start=True,
                                stop=True,
                            )
                        ut_dst = ut_v[:, :, :, tt * 4 : (tt + 1) * 4]
                        ut_src = ups[:].rearrange("p (c h i) -> p c h i", c=NC_DM, h=H)
                        if tt % 2 == 0:
                            nc.scalar.copy(ut_dst, ut_src)
                        else:
                            nc.vector.tensor_copy(ut_dst, ut_src)

                    # --- den roundtrip: [1, (tt,i,h)] -> [128 (tt,i), 16 h] ---
                    nc.sync.dma_start(
                        den_dram[lt].rearrange("(o n) -> o n", o=1), den_sb[:]
                    )
                    den_lt = den_pool.tile([128, H], F32, name=f"{_rep}dl{lt}", tag="dl")
                    nc.sync.dma_start(
                        den_lt[:], den_dram[lt].rearrange("(p h) -> p h", h=H)
                    )
                    rd = den_pool.tile([128, H], F32, name=f"{_rep}rd{lt}", tag="rd")
                    nc.vector.reciprocal(rd[:], den_lt[:])

                    # --- ctx GEMM: [128 l, 1024 (h,dh)] ---
                    ctxps = ctx_psp.tile([128, 1024], F32, name=f"{_rep}cx{lt}", tag="cx")
                    for h in range(H):
                        for c in range(NC_DM):
                            lhsT = ut_v[:, c, h, :]
                            nc.tensor.matmul(
                                ctxps[:, h * 64 : (h + 1) * 64],
                                lhsT,
                                wvn_sb[:, c * 1024 + h * 64 : c * 1024 + (h + 1) * 64],
                                start=(c == 0),
                                stop=(c == NC_DM - 1),
                            )

                    # --- normalize + residual + LN1 ---
                    ctxn = ln_pool.tile([128, 1024], F32, name=f"{_rep}cn{lt}", tag="cn")
                    nc.vector.tensor_tensor(
                        ctxn.rearrange("p (h x) -> p h x", x=DH),
                        ctxps.rearrange("p (h x) -> p h x", x=DH),
                        rd.rearrange("p (h o) -> p h o", o=1).broadcast_to([128, H, DH]),
                        AL.mult,
                    )
                    src_sb = ln_pool.tile([128, 1024], F32, name=f"{_rep}sr{lt}", tag="sr")
                    nc.gpsimd.dma_start(src_sb[:], srcb_in[lt * 128 : (lt + 1) * 128, :])
                    r = ln_pool.tile([128, 1024], F32, name=f"{_rep}r{lt}", tag="r")
                    nc.vector.tensor_tensor(r[:], ctxn[:], src_sb[:], AL.add)
                    rsum = s2_pool.tile([128, 1], F32, name=f"{_rep}rs{lt}", tag="rs")
                    nc.vector.tensor_reduce(rsum[:], r[:], axis=mybir.AxisListType.X, op=AL.add)
                    nmean = s2_pool.tile([128, 1], F32, name=f"{_rep}mn{lt}", tag="mn")
                    nc.vector.tensor_scalar_mul(nmean[:], rsum[:], -1.0 / DM)
                    sq = ln_pool.tile([128, 1024], F32, name=f"{_rep}sq{lt}", tag="xm")
                    ssq = s2_pool.tile([128, 1], F32, name=f"{_rep}sm{lt}", tag="sm")
                    nc.scalar.activation(sq[:], r[:], AF.Square, bias=nmean[:], accum_out=ssq[:])
                    std = s2_pool.tile([128, 1], F32, name=f"{_rep}sd{lt}", tag="sd")
                    nc.scalar.activation(std[:], ssq[:], AF.Sqrt, bias=eps_sb[:], scale=1.0 / DM)
                    rstd = s2_pool.tile([128, 1], F32, name=f"{_rep}rsd{lt}", tag="rsd")
                    nc.vector.reciprocal(rstd[:], std[:])
                    xn = ln_pool.tile([128, 1024], F32, name=f"{_rep}xn{lt}", tag="sr")
                    nc.vector.tensor_scalar(
                        out=xn[:], in0=r[:], scalar1=nmean[:], scalar2=rstd[:],
                        op0=AL.add, op1=AL.mult,
                    )
                    t1 = ln_pool.tile([128, 1024], F32, name=f"{_rep}t1{lt}", tag="cn")
                    nc.vector.tensor_tensor(t1[:], xn[:], g1_rep[:], AL.mult)
                    x = xres.tile([128, 1024], F32, name=f"x{_rep}_{lt}", tag=f"x{lt}")
                    x_tiles.append(x)
                    nc.vector.tensor_tensor(x[:], t1[:], be1_rep[:], AL.add)
                    x_bf = ln_pool.tile([128, 1024], BF, name=f"{_rep}xb{lt}", tag="xb")
                    nc.vector.tensor_copy(x_bf[:], x[:])
                    for c in range(NC_DM):
                        if True:
                            tp = den_psp.tile([128, 128], BF, name=f"{_rep}tp{lt}_{c}", tag="dp")
                            nc.tensor.transpose(tp[:], x_bf[:, c * 128 : (c + 1) * 128], ident[:])
                            nc.scalar.copy(
                                xT_sb[:, c * LC + lt * 128 : c * LC + (lt + 1) * 128], tp[:]
                            )

            # =========== phase C: FFN + LN2 ===========
            with ExitStack() as pc:
                ff_psp = pc.enter_context(tc.tile_pool(name="ff_ps", bufs=3, space="PSUM"))
                o_psp = pc.enter_context(tc.tile_pool(name="o_ps", bufs=1, space="PSUM"))
                w1_pool = pc.enter_context(tc.tile_pool(name="w1_pool", bufs=3))
                ff1_sb = utp.tile([128, NFF * LC], BF, name=f"ff1_{_rep}", tag="ut")
                outps = [
                    o_psp.tile([128, 512], F32, name=f"{_rep}ops{i}", tag=f"ops{i}")
                    for i in range(4)
                ]
                for cc in range(NFF):
                    wwt = w1_pool.tile([128, 2048], BF, name=f"{_rep}ww{cc}", tag="w1t")
                    nc.gpsimd.dma_start(wwt[:], ww_in[cc])
                    w1t = wwt[:, 0:1024]
                    ffps = ff_psp.tile([128, LC], F32, name=f"{_rep}ffps{cc}", tag="ffps")
                    for c in range(NC_DM):
                        nc.tensor.matmul(
                            ffps[:],
                            wwt[:, c * 128 : (c + 1) * 128],
                            xT_sb[:, c * LC : (c + 1) * LC],
                            start=(c == 0),
                            stop=(c == NC_DM - 1),
                        )
                    nc.scalar.activation(
                        ff1_sb[:, cc * LC : (cc + 1) * LC],
                        ffps[:],
                        AF.Gelu,
                        bias=b1_sb[:, cc : cc + 1],
                    )
                    w2t = wwt[:, 1024:2048]
                    for lt in range(NLT):
                        for hh in range(2):
                            nc.tensor.matmul(
                                outps[lt * 2 + hh][:],
                                ff1_sb[:, cc * LC + lt * 128 : cc * LC + (lt + 1) * 128],
                                wwt[:, 1024 + hh * 512 : 1024 + (hh + 1) * 512],
                                start=(cc == 0),
                                stop=(cc == NFF - 1),
                            )

                ln2_pool = pc.enter_context(tc.tile_pool(name="ln2_pool", bufs=2))
                s3_pool = pc.enter_context(tc.tile_pool(name="s3_pool", bufs=2))
                for lt in range(NLT):
                    xb2 = ln2_pool.tile([128, 1024], F32, name=f"{_rep}xb2_{lt}", tag="xb2")
                    nc.vector.tensor_tensor(xb2[:], x_tiles[lt][:], b2_rep[:], AL.add)
                    r2 = ln2_pool.tile([128, 1024], F32, name=f"{_rep}r2_{lt}", tag="r2")
                    for hh in range(2):
                        nc.vector.tensor_tensor(
                            r2[:, hh * 512 : (hh + 1) * 512],
                            xb2[:, hh * 512 : (hh + 1) * 512],
                            outps[lt * 2 + hh][:],
                            AL.add,
                        )
                    rsum2 = s3_pool.tile([128, 1], F32, name=f"{_rep}rsum2_{lt}", tag="rs2")
                    nc.vector.tensor_reduce(rsum2[:], r2[:], axis=mybir.AxisListType.X, op=AL.add)
                    nmean2 = s3_pool.tile([128, 1], F32, name=f"{_rep}mean2_{lt}", tag="mn2")
                    nc.vector.tensor_scalar_mul(nmean2[:], rsum2[:], -1.0 / DM)
                    sq2 = ln2_pool.tile([128, 1024], F32, name=f"{_rep}sq2_{lt}", tag="xm2")
                    ssq2 = s3_pool.tile([128, 1], F32, name=f"{_rep}ssq2_{lt}", tag="sm2")
                    nc.scalar.activation(sq2[:], r2[:], AF.Square, bias=nmean2[:], accum_out=ssq2[:])
                    std2 = s3_pool.tile([128, 1], F32, name=f"{_rep}std2_{lt}", tag="sd2")
                    nc.scalar.activation(std2[:], ssq2[:], AF.Sqrt, bias=eps_sb[:], scale=1.0 / DM)
                    rstd2 = s3_pool.tile([128, 1], F32, name=f"{_rep}rstd2_{lt}", tag="rsd2")
                    nc.vector.reciprocal(rstd2[:], std2[:])
                    xn2 = ln2_pool.tile([128, 1024], F32, name=f"{_rep}xn2_{lt}", tag="xb2")
                    nc.vector.tensor_scalar(
                        out=xn2[:], in0=r2[:], scalar1=nmean2[:], scalar2=rstd2[:],
                        op0=AL.add, op1=AL.mult,
                    )
                    t2 = ln2_pool.tile([128, 1024], F32, name=f"{_rep}t2_{lt}", tag="r2")
                    nc.vector.tensor_tensor(t2[:], xn2[:], g2_rep[:], AL.mult)
                    y = ln2_pool.tile([128, 1024], F32, name=f"{_rep}y{lt}", tag="xm2")
                    nc.vector.tensor_tensor(y[:], t2[:], be2_rep[:], AL.add)
                    nc.sync.dma_start(out[lt * 128 : (lt + 1) * 128, :], y[:])

    nc.compile()
    return nc


def _prep_core(src_c, tgt_c, mask_c, W):
    """Host-side layout prep for one core's shard.  Returns the in_map dict."""
    bf = ml_dtypes.bfloat16
    X = np.ascontiguousarray(tgt_c.reshape(LC * D, DM))

    f8 = ml_dtypes.float8_e4m3
    # xx: [xc | xr]: xc = X^T chunks, xr = natural row tiles (fp8e4m3)
    xx = np.empty((NT, 128, 2048), dtype=f8)
    xx[:, :, 0:1024] = X.reshape(NT, 128, NC_DM, 128).transpose(0, 3, 2, 1).reshape(
        NT, 128, 1024
    ).astype(f8)
    xx[:, :, 1024:2048] = X.reshape(NT, 128, 1024).astype(f8)

    # st: [128, CH*LC]; [p, c*LC+f] = src_c[f, c*128+p]; chunk 8 row0 = ones
    st = np.zeros((128, CH, LC), dtype=np.float32)
    st[:, :NC_DM, :] = src_c.reshape(LC, NC_DM, 128).transpose(2, 1, 0)
    st[0, NC_DM, :] = 1.0

    def wprep(Wm, b, scale=1.0):
        Wp = np.zeros((CH * 128, DM), dtype=np.float32)
        Wp[:DM] = Wm * scale
        Wp[DM] = b * scale
        return np.ascontiguousarray(
            Wp.reshape(CH, 128, DM).transpose(1, 0, 2).reshape(128, CH * 1024)
        ).astype(bf)

    # wkt: [hl*64+p, hp*1024 + c*128 + m] = Wk[c*128+m, (2hp+hl)*64+p]
    wkt = np.ascontiguousarray(
        W["Wk"].reshape(NC_DM, 128, NHP, 2, 64).transpose(3, 4, 2, 0, 1).reshape(128, NHP * NC_DM * 128)
    ).astype(bf)

    wvn = np.ascontiguousarray(
        W["Wv"].reshape(NC_DM, 128, 1024).transpose(1, 0, 2).reshape(128, NC_DM * 1024)
    ).astype(bf)
    ww = np.empty((NFF, 128, 2048), dtype=bf)
    ww[:, :, 0:1024] = W["W1"].reshape(NC_DM, 128, NFF, 128).transpose(2, 1, 0, 3).reshape(
        NFF, 128, 1024
    ).astype(bf)
    ww[:, :, 1024:2048] = W["W2"].reshape(NFF, 128, DM).astype(bf)

    # maskp: [t, i*32+d, i2*16+h] = mask[4t+i, d] if i2==i else -1e30
    maskp = np.full((NT, 4, D, H, 4), -1e30, dtype=np.float32)
    mc = mask_c.reshape(NT, 4, D)
    for i in range(4):
        maskp[:, i, :, :, i] = mc[:, i, :, None]
    maskp = np.ascontiguousarray(
        maskp.reshape(NLT, TPB, 128, 64).transpose(0, 2, 1, 3).reshape(NLT, 128, TPB * 64)
    )

    srcb = np.ascontiguousarray(src_c + W["bv"][None, :]).astype(np.float32)

    return {
        "xx": xx,
        "st": np.ascontiguousarray(st.reshape(128, CH * LC)).astype(bf),
        "wq": wprep(W["Wq"], W["bq"], scale=1.0 / SCALE),
        "wkt": wkt,
        "wvn": wvn,
        "ww": ww,
        "maskp": maskp,
        "srcb": srcb,
        "b1p": np.ascontiguousarray(W["b1"].reshape(NFF, 128).T).astype(np.float32),
        "b2p": W["b2"].reshape(1, DM).astype(np.float32),
        "g1p": W["g1"].reshape(1, DM).astype(np.float32),
        "be1p": W["beta1"].reshape(1, DM).astype(np.float32),
        "g2p": W["g2"].reshape(1, DM).astype(np.float32),
        "be2p": W["beta2"].reshape(1, DM).astype(np.float32),
    }


def make_in_maps(**inputs):
    inp = {k: np.asarray(v) for k, v in inputs.items()}
    W = {
        k: inp[k]
        for k in ("Wq", "bq", "Wk", "bk", "Wv", "bv", "W1", "b1", "W2", "b2",
                  "g1", "beta1", "g2", "beta2")
    }
    in_maps = []
    for c in range(NCORES):
        sl = slice(c * LC, (c + 1) * LC)
        in_maps.append(_prep_core(inp["src"][sl], inp["target"][sl], inp["attn_mask"][sl], W))
    return in_maps


def get_nc(repeat=1):
    key = ("nc", repeat)
    if key not in _CACHE:
        _CACHE[key] = _build_nc(repeat)
    return _CACHE[key]


def kernel(**inputs) -> np.ndarray:
    nc = get_nc()
    in_maps = make_in_maps(**inputs)
    res = run_bass_kernel_spmd(nc, in_maps, core_ids=list(range(NCORES)))
    return np.concatenate([res.results[c]["out"] for c in range(NCORES)], axis=0)
